# revision 1
# baseline (speedup 1.0000x reference)
"""Single-head causal attention (B=8, T=2048, D=1024, H=128) on 8 TRN2 NeuronCores.

Sharding: one batch element per core (data-parallel over B).

Per-core algorithm (bf16 inputs, fp32 PSUM accumulation):
  - host supplies x^T [D, T] (partition-major [128, ND, T]) and weights bf16
  - projections run in two 1024-wide passes (chunks {0,1}, then {2,3}):
    Q^T,K^T,V^T = W^T @ x^T, each d-step as two N=512 matmuls into one
    2-bank [128,1024] PSUM accumulator
  - V^T PE-transposed to V [T, H] bf16 tiles (4 transposes per PSUM tile,
    one evacuation)
  - per 512-wide q-chunk: S^T[k,q] = K^T_tile.T @ Q^T_chunk (bf16), exp via
    ACT to bf16 P^T (no max-subtraction: logits are O(6)), causal mask on
    the 128x128 diagonal block via DVE multiply, O^T[h,q] += V_tile.T @ P^T
    in PSUM, DVE bf16 row-sum partials. The j-loop is software-pipelined
    (S_{j+1} emitted before PV_j so the in-order PE queue never waits on
    the ACT exp of tile j).
  - chunks processed in order 1,3,2,0: the last chunk is the smallest so
    the exp-chain overhang exposed at kernel end is minimal.
  - chunk tails (reciprocal_approx_fast -> gpsimd partition_broadcast ->
    DVE scale -> DMA out) are emitted after the next chunk's body so their
    latency hides under it.
  - constants/masks/ACT-exp-table warm run before the input DMA issues (the
    gpsimd custom-op library load rides the DMA queue); 16 warm-up matmuls
    keep the PE activity monitor from starting the real work at half clock.
  - host transposes O^T -> [T, H] per batch.
"""
import numpy as np

B, T, D, H = 8, 2048, 1024, 128
ND = D // 128      # 8 d-tiles
NTK = T // 128     # 16 k-tiles
NCH = T // 512     # 4 q-chunks
SCALE = float(H) ** -0.5

_CACHE = {}


def _build():
    import concourse.bass as bass  # noqa: F401
    from concourse import bacc
    import concourse.mybir as mybir
    import concourse.tile as tile
    from concourse.masks import make_identity

    f32 = mybir.dt.float32
    bf16 = mybir.dt.bfloat16

    nc = bacc.Bacc("TRN2", target_bir_lowering=False)
    # xt[p, n, t] = x[b].T[n*128 + p, t] — partition-major so a whole range
    # can stream with one 3D DMA
    xt_d = nc.dram_tensor("xt", (128, ND, T), bf16, kind="ExternalInput")
    wq_d = nc.dram_tensor("wq", (128, ND, H), bf16, kind="ExternalInput")
    wk_d = nc.dram_tensor("wk", (128, ND, H), bf16, kind="ExternalInput")
    wv_d = nc.dram_tensor("wv", (128, ND, H), bf16, kind="ExternalInput")
    ot_d = nc.dram_tensor("ot", (H, T), f32, kind="ExternalOutput")

    with tile.TileContext(nc) as tc:
        with (
            tc.tile_pool(name="sb", bufs=1) as sb,
            tc.tile_pool(name="ps", bufs=1, space="PSUM") as ps,
        ):
            # ---- constants first: gpsimd library load + mask builds + ACT
            # table load all happen under the input-DMA shadow ----
            ident = sb.tile([128, 128], bf16, tag="ident")
            make_identity(nc, ident[:])
            # tri32[k, q] = 1 iff q >= k (same mask for every diagonal block)
            tri32 = sb.tile([128, 128], f32, tag="tri32")
            nc.gpsimd.memset(tri32[:], 1.0)
            nc.gpsimd.affine_select(
                out=tri32[:], in_=tri32[:],
                compare_op=mybir.AluOpType.is_ge, fill=0.0,
                base=0, pattern=[[1, 128]], channel_multiplier=-1,
            )
            trimask = sb.tile([128, 128], bf16, tag="trimask")
            nc.vector.tensor_copy(trimask[:], tri32[:])
            ones_c32 = sb.tile([128, 1], f32, tag="ones_c32")
            nc.gpsimd.memset(ones_c32[:], 1.0)
            ones_col = sb.tile([128, 1], bf16, tag="ones_col")
            nc.vector.tensor_copy(ones_col[:], ones_c32[:])
            # warm the ACT exp table while DMA streams in
            warm = sb.tile([128, 1], bf16, tag="warm")
            nc.scalar.activation(warm[:], ones_c32[:],
                                 mybir.ActivationFunctionType.Exp, scale=1.0)

            # ---- loads ----
            wq = sb.tile([128, ND, H], bf16, tag="wq")
            wk = sb.tile([128, ND, H], bf16, tag="wk")
            wv = sb.tile([128, ND, H], bf16, tag="wv")
            xt = sb.tile([128, ND, T], bf16, tag="xt")
            nc.sync.dma_start(wv[:], wv_d[:])
            # first half at d-tile granularity (compute starts asap)
            for d in range(ND):
                nc.sync.dma_start(xt[:, d, 0:1024], xt_d[:, d, 0:1024])
            nc.sync.dma_start(wk[:], wk_d[:])
            nc.sync.dma_start(wq[:], wq_d[:])
            nc.sync.dma_start(xt[:, :, 1024:2048], xt_d[:, :, 1024:2048])

            # warm the PE clock (HAM activity window) while DMA streams in
            wmm = ps.tile([128, 128], f32, tag="otacc", bufs=2)
            for i in range(16):
                nc.tensor.matmul(wmm[:], ident[:], ident[:],
                                 start=(i == 0), stop=(i == 15))

            qt = sb.tile([128, T], bf16, tag="qt")   # Q^T [h, t]
            kt = sb.tile([128, T], bf16, tag="kt")   # K^T [h, t]
            v = sb.tile([128, NTK, H], bf16, tag="v")  # V [k, h] tiles

            def projpass(P):
                """QKV projections for T range [1024P, 1024P+1024) with
                N=1024 matmuls (one LDWEIGHTS per 1024 output columns)."""
                t0 = 1024 * P
                vt = sb.tile([128, 1024], bf16, tag="vt", bufs=2)
                for w_sb, dst, off in ((wv, vt, None), (wk, kt, t0), (wq, qt, t0)):
                    acc = ps.tile([128, 1024], f32, tag="big", bufs=2,
                                  name=f"acc_{P}")
                    for d in range(ND):
                        for hh in range(2):
                            nc.tensor.matmul(
                                acc[:, 512 * hh:512 * (hh + 1)], w_sb[:, d, :],
                                xt[:, d, t0 + 512 * hh:t0 + 512 * (hh + 1)],
                                start=(d == 0), stop=(d == ND - 1),
                            )
                    with nc.allow_low_precision(reason="bf16 qkv"):
                        if off is None:
                            nc.vector.tensor_copy(dst[:], acc[:])
                        else:
                            nc.vector.tensor_copy(dst[:, off:off + 1024], acc[:])
                for h2 in range(2):  # 4 transposes -> one PSUM tile -> one evac
                    tp = ps.tile([128, 512], bf16, tag="stp", bufs=2)
                    for jj in range(4):
                        nc.tensor.transpose(
                            tp[:, jj * 128:(jj + 1) * 128],
                            vt[:, 512 * h2 + jj * 128:512 * h2 + (jj + 1) * 128],
                            ident[:])
                    with nc.allow_low_precision(reason="bf16 v"):
                        nc.vector.tensor_copy(
                            v[:, 8 * P + 4 * h2:8 * P + 4 * h2 + 4, :], tp[:])

            def body(c):
                """Software-pipelined S/exp/PV/pacc loop + row sums."""
                otp = ps.tile([128, 512], f32, tag="otacc", bufs=2)
                pacc = sb.tile([128, 512], bf16, tag="pacc", bufs=2)
                nk = 4 * c + 4

                def lo_of(j):
                    return 128 * (j - 4 * c) if j >= 4 * c else 0

                def emit_s(j):
                    lo = lo_of(j)
                    stp = ps.tile([128, 512], f32, tag="stp", bufs=2)
                    nc.tensor.matmul(
                        stp[:, lo:512],
                        kt[:, j * 128:(j + 1) * 128],
                        qt[:, c * 512 + lo:(c + 1) * 512],
                        start=True, stop=True,
                    )
                    pt = sb.tile([128, 512], bf16, tag="pt", bufs=6)
                    nc.scalar.activation(
                        pt[:, lo:512], stp[:, lo:512],
                        mybir.ActivationFunctionType.Exp, scale=SCALE)
                    if j >= 4 * c:  # diag: zero the upper-left triangle
                        nc.vector.tensor_mul(
                            pt[:, lo:lo + 128], pt[:, lo:lo + 128], trimask[:])
                    return pt

                def emit_pv(j, pt):
                    lo = lo_of(j)
                    nc.tensor.matmul(
                        otp[:, lo:512], v[:, j, :], pt[:, lo:512],
                        start=(j == 0), stop=(j == nk - 1),
                    )
                    with nc.allow_low_precision(reason="bf16 softmax denom"):
                        if j == 0:
                            nc.vector.tensor_copy(pacc[:], pt[:])
                        else:
                            nc.vector.tensor_add(pacc[:, lo:512], pacc[:, lo:512],
                                                 pt[:, lo:512])

                pts = {0: emit_s(0)}
                if nk > 1:
                    pts[1] = emit_s(1)
                for j in range(nk):
                    if j + 2 < nk:
                        pts[j + 2] = emit_s(j + 2)
                    emit_pv(j, pts.pop(j))
                sums = ps.tile([1, 512], f32, tag="otacc", bufs=2)
                nc.tensor.matmul(sums[:], ones_col[:], pacc[:], start=True, stop=True)
                return otp, sums

            def tail(c, otp, sums):
                """normalize + DMA out for chunk c (emitted late: overlaps
                the next chunk's body, so nothing here stalls PE)."""
                recip = sb.tile([1, 512], f32, tag="recip", bufs=2)
                nc.vector.reciprocal_approx_fast(out=recip[:], in_=sums[:])
                bc_sb = sb.tile([128, 512], f32, tag="bcsb", bufs=2)
                nc.gpsimd.partition_broadcast(bc_sb[:], recip[:])
                ot_sb = sb.tile([128, 512], f32, tag="otsb", bufs=2)
                nc.vector.tensor_mul(ot_sb[:], otp[:], bc_sb[:])
                nc.sync.dma_start(ot_d[:, c * 512:(c + 1) * 512], ot_sb[:])

            # ---- pipeline: two proj passes; chunks in order 1,3,2,0 so the
            # smallest chunk (0) ends the kernel with minimal exp overhang;
            # each tail is emitted one body later ----
            projpass(0)
            p1 = (1, *body(1))
            projpass(1)
            tail(*p1)
            p3 = (3, *body(3))
            tail(*p3)
            p2 = (2, *body(2))
            tail(*p2)
            p0 = (0, *body(0))
            tail(*p0)

    nc.compile()
    return nc


def _in_maps(x, W_Q, W_K, W_V):
    import ml_dtypes

    bf16 = ml_dtypes.bfloat16

    def warr(W):
        return np.ascontiguousarray(
            np.asarray(W, np.float32).reshape(ND, 128, H).transpose(1, 0, 2)
        ).astype(bf16)

    wqr, wkr, wvr = warr(W_Q), warr(W_K), warr(W_V)
    x = np.asarray(x, np.float32)
    return [
        {"xt": np.ascontiguousarray(
            x[b].T.reshape(ND, 128, T).transpose(1, 0, 2)).astype(bf16),
         "wq": wqr, "wk": wkr, "wv": wvr}
        for b in range(B)
    ]


def _run(inputs, **kw):
    from concourse import bass_utils

    if "nc" not in _CACHE:
        _CACHE["nc"] = _build()
    return bass_utils.run_bass_kernel_spmd(
        _CACHE["nc"], _in_maps(**inputs), core_ids=list(range(B)), **kw)


def kernel(x, W_Q, W_K, W_V):
    res = _run({"x": x, "W_Q": W_Q, "W_K": W_K, "W_V": W_V})
    return np.stack([res.results[b]["ot"].T for b in range(B)]).astype(np.float32)



# revision 3
# speedup vs baseline: 1.0028x; 1.0028x over previous
"""Single-head causal attention (B=8, T=2048, D=1024, H=128) on 8 TRN2 NeuronCores.

Sharding: one batch element per core (data-parallel over B).

v2 design (per core, bf16 inputs, fp32 PSUM accumulation):
  - packed weights w = [V|K|Q] d-tiles in one DRAM tensor (one DMA); x^T
    streamed as 8 first-half d-tiles (fine-grained gating) + 2 second-half
    block DMAs.
  - projections interleave V/K/Q per d-tile: one PSUM ring (3 slots x
    [128,1024] = 6 banks) holds vacc/kacc/qacc so PE consumes each arriving
    x d-tile once at ~1.3us/tile, matching the HBM stream rate; K/Q evac to
    bf16 SBUF; V PE-transposed to [t,h] tiles via the same ring.
  - attention per 512-wide q-chunk: S^T tiles written at region start
    (diagonal tiles shifted left so a chunk's S area is contiguous), so exp
    needs only 2c+2 ACT calls per chunk (the ACT engine is the attention-
    phase bottleneck at ~0.95ns/col). Causal mask via DVE multiply on the
    [0:128] block of each diagonal region. PV accumulates in PSUM; bf16
    row-sum partials on DVE; denominator via ones-column matmul.
  - unnormalized O^T (bf16) + per-column sums (f32) DMA'd out; the host
    divides and transposes (removes the gpsimd partition-broadcast +
    reciprocal chain from the kernel tail entirely).
  - 40 warmup matmuls on a memset tile keep the PE activity monitor busy
    through the DMA lead-in so real work starts at full clock.
  - order: proj(first half), body0, body1, proj(second half), body2, body3;
    each body's exp/DVE work hides under the next phase's matmuls; tails
    are emitted inline (they are two cheap DVE copies + DMA).
"""
import numpy as np

B, T, D, H = 8, 2048, 1024, 128
ND = D // 128      # 8 d-tiles
NTK = T // 128     # 16 k-tiles
NCH = T // 512     # 4 q-chunks
SCALE = float(H) ** -0.5

_CACHE = {}


def _build():
    import concourse.bass as bass  # noqa: F401
    from concourse import bacc
    import concourse.mybir as mybir
    import concourse.tile as tile
    from concourse.masks import make_identity

    f32 = mybir.dt.float32
    bf16 = mybir.dt.bfloat16

    nc = bacc.Bacc("TRN2", target_bir_lowering=False)
    # xt[p, n, t] = x[b].T[n*128 + p, t] — partition-major
    xt_d = nc.dram_tensor("xt", (128, ND, T), bf16, kind="ExternalInput")
    # w[p, 8o+d, h]: o=0 V, o=1 K, o=2 Q
    w_d = nc.dram_tensor("w", (128, 3 * ND, H), bf16, kind="ExternalInput")
    ot_d = nc.dram_tensor("ot", (H, T), bf16, kind="ExternalOutput")
    sums_d = nc.dram_tensor("sums", (1, T), f32, kind="ExternalOutput")

    with tile.TileContext(nc) as tc:
        with (
            tc.tile_pool(name="sb", bufs=1) as sb,
            tc.tile_pool(name="ps", bufs=1, space="PSUM") as ps,
        ):
            # ---- constants: warmsrc first so PE warmup starts asap; the
            # rest builds under the input-DMA shadow ----
            warmsrc = sb.tile([128, 128], bf16, tag="warmsrc")
            nc.gpsimd.memset(warmsrc[:], 1.0)
            ident = sb.tile([128, 128], bf16, tag="ident")
            make_identity(nc, ident[:])
            # tri32[k, q] = 1 iff q >= k (same mask for every diagonal block)
            tri32 = sb.tile([128, 128], f32, tag="tri32")
            nc.gpsimd.memset(tri32[:], 1.0)
            nc.gpsimd.affine_select(
                out=tri32[:], in_=tri32[:],
                compare_op=mybir.AluOpType.is_ge, fill=0.0,
                base=0, pattern=[[1, 128]], channel_multiplier=-1,
            )
            trimask = sb.tile([128, 128], bf16, tag="trimask")
            nc.vector.tensor_copy(trimask[:], tri32[:])
            ones_col = sb.tile([128, 1], bf16, tag="ones_col")
            nc.gpsimd.memset(ones_col[:], 1.0)
            # warm the ACT exp table while DMA streams in
            warm = sb.tile([128, 1], bf16, tag="warm")
            nc.scalar.activation(warm[:], warmsrc[:, 0:1],
                                 mybir.ActivationFunctionType.Exp, scale=1.0)

            # ---- input DMA launches (single HW queue, completion-order =
            # need-order) ----
            w = sb.tile([128, 3 * ND, H], bf16, tag="w")
            xt = sb.tile([128, ND, T], bf16, tag="xt")
            nc.sync.dma_start(w[:], w_d[:])
            for d in range(ND):
                nc.sync.dma_start(xt[:, d, 0:1024], xt_d[:, d, 0:1024])
            nc.sync.dma_start(xt[:, 0:4, 1024:2048], xt_d[:, 0:4, 1024:2048])
            nc.sync.dma_start(xt[:, 4:8, 1024:2048], xt_d[:, 4:8, 1024:2048])

            # ---- PE warmup: keep the HAM busy through the DMA lead-in ----
            wmm = ps.tile([128, 1024], f32, tag="ring", bufs=3, name="wmm")
            for i in range(40):
                nc.tensor.matmul(wmm[:, 0:128], warmsrc[:], warmsrc[:],
                                 start=(i == 0), stop=(i == 39))

            qt = sb.tile([128, T], bf16, tag="qt")   # Q^T [h, t]
            kt = sb.tile([128, T], bf16, tag="kt")   # K^T [h, t]
            v = sb.tile([128, NTK, H], bf16, tag="v")  # V [k, h] tiles
            sums_sb = sb.tile([1, T], f32, tag="sums_sb")

            def projpass(P):
                """V/K/Q projections for T range [1024P, 1024P+1024),
                interleaved per d-tile so PE tracks the DMA stream."""
                t0 = 1024 * P
                vacc = ps.tile([128, 1024], f32, tag="ring", bufs=3,
                               name=f"vacc{P}")
                kacc = ps.tile([128, 1024], f32, tag="ring", bufs=3,
                               name=f"kacc{P}")
                qacc = ps.tile([128, 1024], f32, tag="ring", bufs=3,
                               name=f"qacc{P}")
                for d in range(ND):
                    st, sp = (d == 0), (d == ND - 1)
                    for o, acc in ((0, vacc), (1, kacc), (2, qacc)):
                        for hh in range(2):
                            nc.tensor.matmul(
                                acc[:, 512 * hh:512 * (hh + 1)],
                                w[:, 8 * o + d, :],
                                xt[:, d, t0 + 512 * hh:t0 + 512 * (hh + 1)],
                                start=st, stop=sp,
                            )
                vt = sb.tile([128, 1024], bf16, tag="vt", bufs=2,
                             name=f"vt{P}")
                with nc.allow_low_precision(reason="bf16 qkv"):
                    # split across engines: V(DVE, feeds PE transposes),
                    # K(ACT copy), Q(DVE) — next body's S waits on K and Q
                    nc.vector.tensor_copy(vt[:], vacc[:])
                    nc.scalar.activation(kt[:, t0:t0 + 1024], kacc[:],
                                         mybir.ActivationFunctionType.Copy)
                    nc.vector.tensor_copy(qt[:, t0:t0 + 1024], qacc[:])
                for h2 in range(2):  # 4 transposes -> one PSUM tile -> evac
                    tp = ps.tile([128, 512], bf16, tag="ring", bufs=3,
                                 name=f"tp{P}_{h2}")
                    for jj in range(4):
                        nc.tensor.transpose(
                            tp[:, jj * 128:(jj + 1) * 128],
                            vt[:, 512 * h2 + jj * 128:512 * h2 + (jj + 1) * 128],
                            ident[:])
                    with nc.allow_low_precision(reason="bf16 v"):
                        nc.vector.tensor_copy(
                            v[:, 8 * P + 4 * h2:8 * P + 4 * h2 + 4, :], tp[:])

            def body(c):
                """Attention for q-chunk c. Units of two k-tiles share one
                [128,1024] S PSUM tile; diagonal tiles write left-shifted so
                each unit's S region is contiguous -> one exp per unit."""
                nk = 4 * c + 4
                otp = ps.tile([128, 512], f32, tag="otacc", bufs=2,
                              name=f"otp{c}")
                pacc = sb.tile([128, 512], bf16, tag="pacc", bufs=2,
                               name=f"pacc{c}")

                # units: (j0, j1); j >= 4c are diagonal (width 512-lo)
                units = [(2 * k, 2 * k + 1) for k in range(2 * c + 2)]

                def geom(j, prev_w):
                    """(region base, width, lo) for tile j given the width
                    of the unit's first region (0 for the first)."""
                    lo = 128 * (j - 4 * c) if j >= 4 * c else 0
                    return prev_w, 512 - lo, lo

                def emit_su(u):
                    j0, j1 = units[u]
                    stp = ps.tile([128, 1024], f32, tag="ring", bufs=3,
                                  name=f"stp{c}_{u}")
                    pt = sb.tile([128, 1024], bf16, tag="pt", bufs=4,
                                 name=f"pt{c}_{u}")
                    base = 0
                    for j in (j0, j1):
                        base, wd, lo = geom(j, base)
                        nc.tensor.matmul(
                            stp[:, base:base + wd],
                            kt[:, j * 128:(j + 1) * 128],
                            qt[:, c * 512 + lo:(c + 1) * 512],
                            start=True, stop=True,
                        )
                        base += wd
                    nc.scalar.activation(
                        pt[:, 0:base], stp[:, 0:base],
                        mybir.ActivationFunctionType.Exp, scale=SCALE)
                    if j1 >= 4 * c:  # diagonal: zero above-diagonal blocks
                        base = 0
                        for j in (j0, j1):
                            base, wd, lo = geom(j, base)
                            nc.vector.tensor_mul(
                                pt[:, base:base + 128],
                                pt[:, base:base + 128], trimask[:])
                            base += wd
                    return pt

                def emit_pv(u, pt):
                    j0, j1 = units[u]
                    base = 0
                    for j in (j0, j1):
                        base, wd, lo = geom(j, base)
                        nc.tensor.matmul(
                            otp[:, lo:512], v[:, j, :], pt[:, base:base + wd],
                            start=(j == 0), stop=(j == nk - 1),
                        )
                        with nc.allow_low_precision(reason="bf16 denom"):
                            if j == 0:
                                nc.vector.tensor_copy(pacc[:], pt[:, 0:512])
                            else:
                                nc.vector.tensor_add(
                                    pacc[:, lo:512], pacc[:, lo:512],
                                    pt[:, base:base + wd])
                        base += wd

                U = len(units)
                pts = {}
                for u in range(min(2, U)):
                    pts[u] = emit_su(u)
                for u in range(U):
                    if u + 2 < U:
                        pts[u + 2] = emit_su(u + 2)
                    emit_pv(u, pts.pop(u))
                sums = ps.tile([1, 512], f32, tag="ring", bufs=3,
                               name=f"sums{c}")
                nc.tensor.matmul(sums[:], ones_col[:], pacc[:],
                                 start=True, stop=True)
                # tail: stage sums, cast O^T to bf16, DMA out
                nc.vector.tensor_copy(sums_sb[:, c * 512:(c + 1) * 512],
                                      sums[:])
                ot_sb = sb.tile([128, 512], bf16, tag="otsb", bufs=2,
                                name=f"otsb{c}")
                with nc.allow_low_precision(reason="bf16 unnormalized out"):
                    nc.vector.tensor_copy(ot_sb[:], otp[:])
                nc.sync.dma_start(ot_d[:, c * 512:(c + 1) * 512], ot_sb[:])

            projpass(0)
            body(0)
            body(1)
            projpass(1)
            body(2)
            body(3)
            nc.sync.dma_start(sums_d[:], sums_sb[:])

    nc.compile()
    return nc


def _in_maps(x, W_Q, W_K, W_V):
    import ml_dtypes

    bf16 = ml_dtypes.bfloat16

    def warr(W):
        return np.asarray(W, np.float32).reshape(ND, 128, H).transpose(1, 0, 2)

    wr = np.ascontiguousarray(
        np.concatenate([warr(W_V), warr(W_K), warr(W_Q)], axis=1)
    ).astype(bf16)
    x = np.asarray(x, np.float32)
    return [
        {"xt": np.ascontiguousarray(
            x[b].T.reshape(ND, 128, T).transpose(1, 0, 2)).astype(bf16),
         "w": wr}
        for b in range(B)
    ]


def _run(inputs, **kw):
    from concourse import bass_utils

    if "nc" not in _CACHE:
        _CACHE["nc"] = _build()
    return bass_utils.run_bass_kernel_spmd(
        _CACHE["nc"], _in_maps(**inputs), core_ids=list(range(B)), **kw)


def kernel(x, W_Q, W_K, W_V):
    res = _run({"x": x, "W_Q": W_Q, "W_K": W_K, "W_V": W_V})
    out = np.empty((B, T, H), np.float32)
    for b in range(B):
        ot = np.asarray(res.results[b]["ot"], np.float32)   # [H, T]
        s = np.asarray(res.results[b]["sums"], np.float32)  # [1, T]
        out[b] = (ot / s).T
    return out


# revision 11
# speedup vs baseline: 1.0741x; 1.0711x over previous
"""Single-head causal attention (B=8, T=2048, D=1024, H=128) on 8 TRN2 NeuronCores.

Sharding: one batch element per core (data-parallel over B).

v2 design (per core, bf16 inputs, fp32 PSUM accumulation):
  - packed weights w = [V|K|Q] d-tiles in one DRAM tensor (one DMA); x^T
    streamed as 8 first-half d-tiles (fine-grained gating) + 2 second-half
    block DMAs.
  - projections interleave V/K/Q per d-tile: one PSUM ring (3 slots x
    [128,1024] = 6 banks) holds vacc/kacc/qacc so PE consumes each arriving
    x d-tile once at ~1.3us/tile, matching the HBM stream rate; K/Q evac to
    bf16 SBUF; V PE-transposed to [t,h] tiles via the same ring.
  - attention per 512-wide q-chunk: S^T tiles written at region start
    (diagonal tiles shifted left so a chunk's S area is contiguous), so exp
    needs only 2c+2 ACT calls per chunk (the ACT engine is the attention-
    phase bottleneck at ~0.95ns/col). Causal mask via DVE multiply on the
    [0:128] block of each diagonal region. PV accumulates in PSUM; bf16
    row-sum partials on DVE; denominator via ones-column matmul.
  - unnormalized O^T (bf16) + per-column sums (f32) DMA'd out; the host
    divides and transposes (removes the gpsimd partition-broadcast +
    reciprocal chain from the kernel tail entirely).
  - 40 warmup matmuls on a memset tile keep the PE activity monitor busy
    through the DMA lead-in so real work starts at full clock.
  - order: proj(first half), body0, body1, proj(second half), body2, body3;
    each body's exp/DVE work hides under the next phase's matmuls; tails
    are emitted inline (they are two cheap DVE copies + DMA).
"""
import numpy as np

B, T, D, H = 8, 2048, 1024, 128
ND = D // 128      # 8 d-tiles
NTK = T // 128     # 16 k-tiles
NCH = T // 512     # 4 q-chunks
SCALE = float(H) ** -0.5

_CACHE = {}


def _build():
    import concourse.bass as bass  # noqa: F401
    from concourse import bacc
    import concourse.mybir as mybir
    import concourse.tile as tile
    from concourse.masks import make_identity

    f32 = mybir.dt.float32
    bf16 = mybir.dt.bfloat16

    nc = bacc.Bacc("TRN2", target_bir_lowering=False)
    # xt[p, n, t] = x[b].T[n*128 + p, t] — partition-major
    xt_d = nc.dram_tensor("xt", (128, ND, T), bf16, kind="ExternalInput")
    # w[p, 8o+d, h]: o=0 V, o=1 K, o=2 Q
    w_d = nc.dram_tensor("w", (128, 3 * ND, H), bf16, kind="ExternalInput")
    ot_d = nc.dram_tensor("ot", (H, T), bf16, kind="ExternalOutput")
    sums_d = nc.dram_tensor("sums", (1, T), f32, kind="ExternalOutput")

    with tile.TileContext(nc) as tc:
        with (
            tc.tile_pool(name="sb", bufs=1) as sb,
            tc.tile_pool(name="ps", bufs=1, space="PSUM") as ps,
        ):
            # ---- constants: warmsrc first so PE warmup starts asap; the
            # rest builds under the input-DMA shadow ----
            warmsrc = sb.tile([128, 128], bf16, tag="warmsrc")
            nc.gpsimd.memset(warmsrc[:], 1.0)
            ident = sb.tile([128, 128], bf16, tag="ident")
            make_identity(nc, ident[:])
            # tri32[k, q] = 1 iff q >= k (same mask for every diagonal block)
            tri32 = sb.tile([128, 128], f32, tag="tri32")
            nc.gpsimd.memset(tri32[:], 1.0)
            nc.gpsimd.affine_select(
                out=tri32[:], in_=tri32[:],
                compare_op=mybir.AluOpType.is_ge, fill=0.0,
                base=0, pattern=[[1, 128]], channel_multiplier=-1,
            )
            trimask = sb.tile([128, 128], bf16, tag="trimask")
            nc.vector.tensor_copy(trimask[:], tri32[:])
            ones_col = sb.tile([128, 1], bf16, tag="ones_col")
            nc.gpsimd.memset(ones_col[:], 1.0)
            # warm the ACT exp table while DMA streams in
            warm = sb.tile([128, 1], bf16, tag="warm")
            nc.scalar.activation(warm[:], warmsrc[:, 0:1],
                                 mybir.ActivationFunctionType.Exp, scale=1.0)

            # ---- input DMA launches (single HW queue, completion-order =
            # need-order: wv before d0 so the first V matmul fires asap;
            # h1 second-half quarters first so the P1a pass unblocks early) ----
            w = sb.tile([128, 3 * ND, H], bf16, tag="w")
            xt = sb.tile([128, ND, T], bf16, tag="xt")
            nc.sync.dma_start(w[:, 0:8, :], w_d[:, 0:8, :])
            nc.sync.dma_start(xt[:, 0, 0:1024], xt_d[:, 0, 0:1024])
            nc.sync.dma_start(w[:, 8:16, :], w_d[:, 8:16, :])
            nc.sync.dma_start(xt[:, 1, 0:1024], xt_d[:, 1, 0:1024])
            nc.sync.dma_start(w[:, 16:24, :], w_d[:, 16:24, :])
            for d in range(2, ND):
                nc.sync.dma_start(xt[:, d, 0:1024], xt_d[:, d, 0:1024])
            nc.sync.dma_start(xt[:, 0:4, 1536:2048], xt_d[:, 0:4, 1536:2048])
            nc.sync.dma_start(xt[:, 4:8, 1536:2048], xt_d[:, 4:8, 1536:2048])
            nc.sync.dma_start(xt[:, 0:4, 1024:1536], xt_d[:, 0:4, 1024:1536])
            nc.sync.dma_start(xt[:, 4:8, 1024:1536], xt_d[:, 4:8, 1024:1536])

            # ---- PE warmup: keep the HAM busy through the DMA lead-in ----
            wmm = ps.tile([128, 1024], f32, tag="ring", bufs=3, name="wmm")
            for i in range(28):
                nc.tensor.matmul(wmm[:, 0:128], warmsrc[:], warmsrc[:],
                                 start=(i == 0), stop=(i == 27))

            qt = sb.tile([128, T], bf16, tag="qt")   # Q^T [h, t]
            kt = sb.tile([128, T], bf16, tag="kt")   # K^T [h, t]
            v = sb.tile([128, NTK, H], bf16, tag="v")  # V [k, h] tiles
            sums_sb = sb.tile([1, T], f32, tag="sums_sb")

            def projpass(P):
                """V/K/Q projections for T range [1024P, 1024P+1024),
                interleaved per d-tile so PE tracks the DMA stream."""
                t0 = 1024 * P
                vacc = ps.tile([128, 1024], f32, tag="ring", bufs=3,
                               name=f"vacc{P}")
                kacc = ps.tile([128, 1024], f32, tag="ring", bufs=3,
                               name=f"kacc{P}")
                qacc = ps.tile([128, 1024], f32, tag="ring", bufs=3,
                               name=f"qacc{P}")
                for d in range(ND):
                    st, sp = (d == 0), (d == ND - 1)
                    for o, acc in ((0, vacc), (1, kacc), (2, qacc)):
                        for hh in range(2):
                            nc.tensor.matmul(
                                acc[:, 512 * hh:512 * (hh + 1)],
                                w[:, 8 * o + d, :],
                                xt[:, d, t0 + 512 * hh:t0 + 512 * (hh + 1)],
                                start=st, stop=sp,
                            )
                vt = sb.tile([128, 1024], bf16, tag="vt", bufs=2,
                             name=f"vt{P}")
                with nc.allow_low_precision(reason="bf16 qkv"):
                    # split across engines: V(DVE, feeds PE transposes),
                    # K(ACT copy), Q(DVE) — next body's S waits on K and Q;
                    # halves so the next body's first S starts on the early
                    # half
                    nc.vector.tensor_copy(vt[:], vacc[:])
                    for hh in range(2):
                        s = slice(512 * hh, 512 * (hh + 1))
                        nc.scalar.activation(
                            kt[:, t0 + 512 * hh:t0 + 512 * (hh + 1)],
                            kacc[:, s], mybir.ActivationFunctionType.Copy)
                        nc.vector.tensor_copy(
                            qt[:, t0 + 512 * hh:t0 + 512 * (hh + 1)],
                            qacc[:, s])
                for h2 in range(2):  # 4 transposes -> one PSUM tile -> evac
                    tp = ps.tile([128, 512], bf16, tag="ring", bufs=3,
                                 name=f"tp{P}_{h2}")
                    for jj in range(4):
                        nc.tensor.transpose(
                            tp[:, jj * 128:(jj + 1) * 128],
                            vt[:, 512 * h2 + jj * 128:512 * h2 + (jj + 1) * 128],
                            ident[:])
                    with nc.allow_low_precision(reason="bf16 v"):
                        nc.vector.tensor_copy(
                            v[:, 8 * P + 4 * h2:8 * P + 4 * h2 + 4, :], tp[:])

            def projpass512(tlo):
                """V/K/Q projections for one 512-col t range."""
                vacc = ps.tile([128, 512], f32, tag="ring", bufs=3,
                               name=f"vacc_{tlo}")
                kacc = ps.tile([128, 512], f32, tag="ring", bufs=3,
                               name=f"kacc_{tlo}")
                qacc = ps.tile([128, 512], f32, tag="ring", bufs=3,
                               name=f"qacc_{tlo}")
                # output-major: not DMA-gated (h1 is resident by now), and it
                # delays the qacc writes past the ring-slot WAR on in-flight
                # body-3 exps
                for o, acc in ((0, vacc), (1, kacc), (2, qacc)):
                    for d in range(ND):
                        nc.tensor.matmul(
                            acc[:], w[:, 8 * o + d, :],
                            xt[:, d, tlo:tlo + 512],
                            start=(d == 0), stop=(d == ND - 1))
                vt = sb.tile([128, 512], bf16, tag="vt", bufs=2,
                             name=f"vt_{tlo}")
                with nc.allow_low_precision(reason="bf16 qkv"):
                    nc.vector.tensor_copy(vt[:], vacc[:])
                    nc.scalar.activation(kt[:, tlo:tlo + 512], kacc[:],
                                         mybir.ActivationFunctionType.Copy)
                    nc.vector.tensor_copy(qt[:, tlo:tlo + 512], qacc[:])
                tp = ps.tile([128, 512], bf16, tag="ring", bufs=3,
                             name=f"tp_{tlo}")
                for jj in range(4):
                    nc.tensor.transpose(
                        tp[:, jj * 128:(jj + 1) * 128],
                        vt[:, jj * 128:(jj + 1) * 128], ident[:])
                with nc.allow_low_precision(reason="bf16 v"):
                    nc.vector.tensor_copy(
                        v[:, tlo // 128:tlo // 128 + 4, :], tp[:])

            def body(c, hook_at=None, hook=None):
                """Attention for q-chunk c. Units of two k-tiles share one
                [128,1024] S PSUM tile; diagonal tiles write left-shifted so
                each unit's S region is contiguous -> one exp per unit."""
                nk = 4 * c + 4
                otp = ps.tile([128, 512], f32, tag="otacc", bufs=2,
                              name=f"otp{c}")
                pacc = sb.tile([128, 512], bf16, tag="pacc", bufs=2,
                               name=f"pacc{c}")

                # units: (j0, j1); j >= 4c are diagonal (width 512-lo)
                units = [(2 * k, 2 * k + 1) for k in range(2 * c + 2)]

                def geom(j, prev_w):
                    """(region base, width, lo) for tile j given the width
                    of the unit's first region (0 for the first)."""
                    lo = 128 * (j - 4 * c) if j >= 4 * c else 0
                    return prev_w, 512 - lo, lo

                def emit_su(u):
                    j0, j1 = units[u]
                    stp = ps.tile([128, 1024], f32, tag="ring", bufs=3,
                                  name=f"stp{c}_{u}")
                    pt = sb.tile([128, 1024], bf16, tag="pt", bufs=4,
                                 name=f"pt{c}_{u}")
                    base = 0
                    for j in (j0, j1):
                        base, wd, lo = geom(j, base)
                        nc.tensor.matmul(
                            stp[:, base:base + wd],
                            kt[:, j * 128:(j + 1) * 128],
                            qt[:, c * 512 + lo:(c + 1) * 512],
                            start=True, stop=True,
                        )
                        base += wd
                    nc.scalar.activation(
                        pt[:, 0:base], stp[:, 0:base],
                        mybir.ActivationFunctionType.Exp, scale=SCALE)
                    if j1 >= 4 * c:  # diagonal: zero above-diagonal blocks
                        base = 0
                        for j in (j0, j1):
                            base, wd, lo = geom(j, base)
                            nc.vector.tensor_mul(
                                pt[:, base:base + 128],
                                pt[:, base:base + 128], trimask[:])
                            base += wd
                    return pt

                def emit_pv(u, pt):
                    j0, j1 = units[u]
                    base = 0
                    for j in (j0, j1):
                        base, wd, lo = geom(j, base)
                        nc.tensor.matmul(
                            otp[:, lo:512], v[:, j, :], pt[:, base:base + wd],
                            start=(j == 0), stop=(j == nk - 1),
                        )
                        with nc.allow_low_precision(reason="bf16 denom"):
                            if j == 0:
                                nc.vector.tensor_copy(pacc[:], pt[:, 0:512])
                            else:
                                nc.vector.tensor_add(
                                    pacc[:, lo:512], pacc[:, lo:512],
                                    pt[:, base:base + wd])
                        base += wd

                U = len(units)
                pts = {}
                for u in range(min(2, U)):
                    pts[u] = emit_su(u)
                for u in range(U):
                    if u == hook_at:
                        hook()
                    if u + 2 < U:
                        pts[u + 2] = emit_su(u + 2)
                    emit_pv(u, pts.pop(u))
                return otp, pacc

            def tail(c, otp, pacc):
                """Denominator + output staging for chunk c. Emitted one
                phase late so the sums matmul (which waits on the DVE pacc
                adds) never blocks the in-order PE queue at a body boundary."""
                sums = ps.tile([1, 512], f32, tag="ring", bufs=3,
                               name=f"sums{c}")
                nc.tensor.matmul(sums[:], ones_col[:], pacc[:],
                                 start=True, stop=True)
                nc.vector.tensor_copy(sums_sb[:, c * 512:(c + 1) * 512],
                                      sums[:])
                ot_sb = sb.tile([128, 512], bf16, tag="otsb", bufs=2,
                                name=f"otsb{c}")
                with nc.allow_low_precision(reason="bf16 unnormalized out"):
                    nc.vector.tensor_copy(ot_sb[:], otp[:])
                nc.sync.dma_start(ot_d[:, c * 512:(c + 1) * 512], ot_sb[:])

            projpass(0)
            b0 = body(0)
            b1 = body(1)
            tail(0, *b0)
            projpass512(1536)      # P1a: Q chunk 3 + K/V tiles 12-15 early
            b1_tail_state = b1

            def mid_hook():
                # P1b's matmuls run while body-3's early exps drain on ACT
                projpass512(1024)
                tail(1, *b1_tail_state)

            b3 = body(3, hook_at=2, hook=mid_hook)
            b2 = body(2)
            tail(3, *b3)
            tail(2, *b2)
            nc.sync.dma_start(sums_d[:], sums_sb[:])

    nc.compile()
    return nc


def _in_maps(x, W_Q, W_K, W_V):
    import ml_dtypes

    bf16 = ml_dtypes.bfloat16

    def warr(W):
        return np.asarray(W, np.float32).reshape(ND, 128, H).transpose(1, 0, 2)

    wr = np.ascontiguousarray(
        np.concatenate([warr(W_V), warr(W_K), warr(W_Q)], axis=1)
    ).astype(bf16)
    x = np.asarray(x, np.float32)
    return [
        {"xt": np.ascontiguousarray(
            x[b].T.reshape(ND, 128, T).transpose(1, 0, 2)).astype(bf16),
         "w": wr}
        for b in range(B)
    ]


def _run(inputs, **kw):
    from concourse import bass_utils

    if "nc" not in _CACHE:
        _CACHE["nc"] = _build()
    return bass_utils.run_bass_kernel_spmd(
        _CACHE["nc"], _in_maps(**inputs), core_ids=list(range(B)), **kw)


def kernel(x, W_Q, W_K, W_V):
    res = _run({"x": x, "W_Q": W_Q, "W_K": W_K, "W_V": W_V})
    out = np.empty((B, T, H), np.float32)
    for b in range(B):
        ot = np.asarray(res.results[b]["ot"], np.float32)   # [H, T]
        s = np.asarray(res.results[b]["sums"], np.float32)  # [1, T]
        out[b] = (ot / s).T
    return out


# revision 12
# speedup vs baseline: 1.1015x; 1.0255x over previous
"""Single-head causal attention (B=8, T=2048, D=1024, H=128) on 8 TRN2 NeuronCores.

Sharding: one batch element per core (data-parallel over B).

v5 design (per core, bf16 inputs, fp32 PSUM accumulation):
  - packed weights w = [V|K|Q] d-tiles, DMA'd as three slices interleaved
    with the first x d-tiles; x^T first half streamed per d-tile, second
    half as quarter blocks ordered so the late projection groups unblock
    in need-order.
  - projections run as 512-col groups (g0..g3), each 8 d-steps of a V/K/Q
    matmul trio into three 1-bank PSUM accs. g0 tracks the HBM stream;
    g1 and chunk-3's Q run inline; the remaining groups are chopped into
    per-d closures and EMITTED AS FILLER between attention units, so the
    PE chews projection work exactly where the ACT-bound attention stretch
    would otherwise idle it, and the ACT exp stream (the attention-phase
    bottleneck at ~0.95ns/col) never waits on a monolithic proj pass.
  - attention per 512-wide q-chunk: two k-tiles per unit share a
    [128,1024] S PSUM tile; diagonal tiles write left-shifted so each
    unit's S area is contiguous -> ONE exp per unit (2c+2 ACT calls per
    chunk). Causal mask via DVE multiply on the first 128 cols of each
    diagonal region. Chunk order 0,1,3,2 with chunk-3 fed by the early Q
    pass. PV accumulates into a single PSUM bank; the O^T bf16 cast is
    emitted inline at body end (frees the bank), the denominator matmul
    one body later (its DVE-dependent wait never blocks the PE queue).
  - unnormalized O^T (bf16) + per-column sums (f32) DMA'd out; the host
    divides and transposes.
  - 40 warmup matmuls bridge the DMA lead-in so the HAM un-throttles the
    PE clock before real work starts and never re-throttles.
  - PSUM banks: S-ring 2x[128,1024]=4, vacc/kacc/qacc 3 (also host the
    V-transpose tiles and denominator rows), otp 1 -> exactly 8.
"""
import numpy as np

B, T, D, H = 8, 2048, 1024, 128
ND = D // 128      # 8 d-tiles
NTK = T // 128     # 16 k-tiles
NCH = T // 512     # 4 q-chunks
SCALE = float(H) ** -0.5

_CACHE = {}


def _build():
    import concourse.bass as bass  # noqa: F401
    from concourse import bacc
    import concourse.mybir as mybir
    import concourse.tile as tile
    from concourse.masks import make_identity

    f32 = mybir.dt.float32
    bf16 = mybir.dt.bfloat16

    nc = bacc.Bacc("TRN2", target_bir_lowering=False)
    xt_d = nc.dram_tensor("xt", (128, ND, T), bf16, kind="ExternalInput")
    # w[p, 8o+d, h]: o=0 V, o=1 K, o=2 Q
    w_d = nc.dram_tensor("w", (128, 3 * ND, H), bf16, kind="ExternalInput")
    ot_d = nc.dram_tensor("ot", (H, T), bf16, kind="ExternalOutput")
    sums_d = nc.dram_tensor("sums", (1, T), f32, kind="ExternalOutput")

    with tile.TileContext(nc) as tc:
        with (
            tc.tile_pool(name="sb", bufs=1) as sb,
            tc.tile_pool(name="ps", bufs=1, space="PSUM") as ps,
        ):
            # ---- constants ----
            warmsrc = sb.tile([128, 128], bf16, tag="warmsrc")
            nc.gpsimd.memset(warmsrc[:], 1.0)
            ident = sb.tile([128, 128], bf16, tag="ident")
            make_identity(nc, ident[:])
            tri32 = sb.tile([128, 128], f32, tag="tri32")
            nc.gpsimd.memset(tri32[:], 1.0)
            nc.gpsimd.affine_select(
                out=tri32[:], in_=tri32[:],
                compare_op=mybir.AluOpType.is_ge, fill=0.0,
                base=0, pattern=[[1, 128]], channel_multiplier=-1,
            )
            trimask = sb.tile([128, 128], bf16, tag="trimask")
            nc.vector.tensor_copy(trimask[:], tri32[:])
            ones_col = sb.tile([128, 1], bf16, tag="ones_col")
            nc.gpsimd.memset(ones_col[:], 1.0)
            warm = sb.tile([128, 1], bf16, tag="warm")
            nc.scalar.activation(warm[:], warmsrc[:, 0:1],
                                 mybir.ActivationFunctionType.Exp, scale=1.0)

            # ---- input DMA launches (need-order on one HW queue) ----
            w = sb.tile([128, 3 * ND, H], bf16, tag="w")
            xt = sb.tile([128, ND, T], bf16, tag="xt")
            nc.sync.dma_start(w[:, 0:8, :], w_d[:, 0:8, :])
            nc.sync.dma_start(xt[:, 0, 0:1024], xt_d[:, 0, 0:1024])
            nc.sync.dma_start(w[:, 8:16, :], w_d[:, 8:16, :])
            nc.sync.dma_start(xt[:, 1, 0:1024], xt_d[:, 1, 0:1024])
            nc.sync.dma_start(w[:, 16:24, :], w_d[:, 16:24, :])
            for d in range(2, ND):
                nc.sync.dma_start(xt[:, d, 0:1024], xt_d[:, d, 0:1024])
            nc.sync.dma_start(xt[:, 0:4, 1536:2048], xt_d[:, 0:4, 1536:2048])
            nc.sync.dma_start(xt[:, 4:8, 1536:2048], xt_d[:, 4:8, 1536:2048])
            nc.sync.dma_start(xt[:, 0:4, 1024:1536], xt_d[:, 0:4, 1024:1536])
            nc.sync.dma_start(xt[:, 4:8, 1024:1536], xt_d[:, 4:8, 1024:1536])

            # ---- PE warmup across the DMA lead-in ----
            wmm = ps.tile([128, 1024], f32, tag="ring", bufs=2, name="wmm")
            for i in range(40):
                nc.tensor.matmul(wmm[:, 0:128], warmsrc[:], warmsrc[:],
                                 start=(i == 0), stop=(i == 39))

            qt = sb.tile([128, T], bf16, tag="qt")
            kt = sb.tile([128, T], bf16, tag="kt")
            v = sb.tile([128, NTK, H], bf16, tag="v")
            sums_sb = sb.tile([1, T], f32, tag="sums_sb")

            OUTS = {"v": 0, "k": 1, "q": 2}
            ACCTAG = {"v": "vacc", "k": "kacc", "q": "qacc"}

            def group_accs(tlo, outs):
                return {o: ps.tile([128, 512], f32, tag=ACCTAG[o], bufs=1,
                                   name=f"{o}acc_{tlo}") for o in outs}

            def group_trio(tlo, accs, d, outs):
                for o in outs:
                    nc.tensor.matmul(
                        accs[o][:], w[:, 8 * OUTS[o] + d, :],
                        xt[:, d, tlo:tlo + 512],
                        start=(d == 0), stop=(d == ND - 1))

            def group_evac(tlo, accs, outs):
                with nc.allow_low_precision(reason="bf16 qkv"):
                    if "k" in outs:
                        nc.scalar.activation(
                            kt[:, tlo:tlo + 512], accs["k"][:],
                            mybir.ActivationFunctionType.Copy)
                    if "q" in outs:
                        nc.vector.tensor_copy(qt[:, tlo:tlo + 512],
                                              accs["q"][:])
                    if "v" in outs:
                        vt = sb.tile([128, 512], bf16, tag="vt", bufs=2,
                                     name=f"vt_{tlo}")
                        nc.vector.tensor_copy(vt[:], accs["v"][:])
                        tp = ps.tile([128, 512], bf16, tag="vacc", bufs=1,
                                     name=f"tp_{tlo}")
                        for jj in range(4):
                            nc.tensor.transpose(
                                tp[:, jj * 128:(jj + 1) * 128],
                                vt[:, jj * 128:(jj + 1) * 128], ident[:])
                        nc.vector.tensor_copy(
                            v[:, tlo // 128:tlo // 128 + 4, :], tp[:])

            def group_inline(tlo, outs=("v", "k", "q")):
                accs = group_accs(tlo, outs)
                for d in range(ND):
                    group_trio(tlo, accs, d, outs)
                group_evac(tlo, accs, outs)

            def group_filler(tlo, outs=("v", "k", "q")):
                """Per-d closures for interleaving into attention bodies."""
                accs = group_accs(tlo, outs)
                fns = [lambda d=d: group_trio(tlo, accs, d, outs)
                       for d in range(ND)]
                fns.append(lambda: group_evac(tlo, accs, outs))
                return fns

            FIL = []

            def pump(n):
                for _ in range(min(n, len(FIL))):
                    FIL.pop(0)()

            def body(c, pump_n):
                nk = 4 * c + 4
                otp = ps.tile([128, 512], f32, tag="otacc", bufs=1,
                              name=f"otp{c}")
                pacc = sb.tile([128, 512], bf16, tag="pacc", bufs=2,
                               name=f"pacc{c}")
                units = [(2 * k, 2 * k + 1) for k in range(2 * c + 2)]

                def geom(j, prev_w):
                    lo = 128 * (j - 4 * c) if j >= 4 * c else 0
                    return prev_w, 512 - lo, lo

                def emit_su(u):
                    j0, j1 = units[u]
                    stp = ps.tile([128, 1024], f32, tag="ring", bufs=2,
                                  name=f"stp{c}_{u}")
                    pt = sb.tile([128, 1024], bf16, tag="pt", bufs=4,
                                 name=f"pt{c}_{u}")
                    base = 0
                    for j in (j0, j1):
                        base, wd, lo = geom(j, base)
                        nc.tensor.matmul(
                            stp[:, base:base + wd],
                            kt[:, j * 128:(j + 1) * 128],
                            qt[:, c * 512 + lo:(c + 1) * 512],
                            start=True, stop=True,
                        )
                        base += wd
                    nc.scalar.activation(
                        pt[:, 0:base], stp[:, 0:base],
                        mybir.ActivationFunctionType.Exp, scale=SCALE)
                    if j1 >= 4 * c:
                        base = 0
                        for j in (j0, j1):
                            base, wd, lo = geom(j, base)
                            nc.vector.tensor_mul(
                                pt[:, base:base + 128],
                                pt[:, base:base + 128], trimask[:])
                            base += wd
                    return pt

                def emit_pv(u, pt):
                    j0, j1 = units[u]
                    base = 0
                    for j in (j0, j1):
                        base, wd, lo = geom(j, base)
                        nc.tensor.matmul(
                            otp[:, lo:512], v[:, j, :], pt[:, base:base + wd],
                            start=(j == 0), stop=(j == nk - 1),
                        )
                        with nc.allow_low_precision(reason="bf16 denom"):
                            if j == 0:
                                nc.vector.tensor_copy(pacc[:], pt[:, 0:512])
                            else:
                                nc.vector.tensor_add(
                                    pacc[:, lo:512], pacc[:, lo:512],
                                    pt[:, base:base + wd])
                        base += wd

                U = len(units)
                pts = {}
                for u in range(min(2, U)):
                    pts[u] = emit_su(u)
                    pump(pump_n)
                for u in range(U):
                    if u + 2 < U:
                        pts[u + 2] = emit_su(u + 2)
                        pump(pump_n)
                    emit_pv(u, pts.pop(u))
                # inline: cast O^T to bf16 (frees the single otp bank) + DMA
                ot_sb = sb.tile([128, 512], bf16, tag="otsb", bufs=2,
                                name=f"otsb{c}")
                with nc.allow_low_precision(reason="bf16 unnormalized out"):
                    nc.vector.tensor_copy(ot_sb[:], otp[:])
                nc.sync.dma_start(ot_d[:, c * 512:(c + 1) * 512], ot_sb[:])
                return pacc

            def tail_sums(c, pacc):
                """Denominator for chunk c, emitted one body late so its
                wait on the DVE pacc adds never blocks the PE queue."""
                sums = ps.tile([1, 512], f32, tag="kacc", bufs=1,
                               name=f"sums{c}")
                nc.tensor.matmul(sums[:], ones_col[:], pacc[:],
                                 start=True, stop=True)
                nc.vector.tensor_copy(sums_sb[:, c * 512:(c + 1) * 512],
                                      sums[:])

            # ---- schedule ----
            group_inline(0)            # g0: DMA-gated
            group_inline(512)          # g1
            group_inline(1536, outs=("q",))   # Q chunk 3 early
            FIL.extend(group_filler(1536, outs=("v", "k")))
            p0 = body(0, pump_n=1)
            p1 = body(1, pump_n=3)
            tail_sums(0, p0)
            FIL.extend(group_filler(1024))
            p3 = body(3, pump_n=3)
            tail_sums(1, p1)
            p2 = body(2, pump_n=2)
            pump(len(FIL))
            tail_sums(3, p3)
            tail_sums(2, p2)
            nc.sync.dma_start(sums_d[:], sums_sb[:])

    nc.compile()
    return nc


def _in_maps(x, W_Q, W_K, W_V):
    import ml_dtypes

    bf16 = ml_dtypes.bfloat16

    def warr(W):
        return np.asarray(W, np.float32).reshape(ND, 128, H).transpose(1, 0, 2)

    wr = np.ascontiguousarray(
        np.concatenate([warr(W_V), warr(W_K), warr(W_Q)], axis=1)
    ).astype(bf16)
    x = np.asarray(x, np.float32)
    return [
        {"xt": np.ascontiguousarray(
            x[b].T.reshape(ND, 128, T).transpose(1, 0, 2)).astype(bf16),
         "w": wr}
        for b in range(B)
    ]


def _run(inputs, **kw):
    from concourse import bass_utils

    if "nc" not in _CACHE:
        _CACHE["nc"] = _build()
    return bass_utils.run_bass_kernel_spmd(
        _CACHE["nc"], _in_maps(**inputs), core_ids=list(range(B)), **kw)


def kernel(x, W_Q, W_K, W_V):
    res = _run({"x": x, "W_Q": W_Q, "W_K": W_K, "W_V": W_V})
    out = np.empty((B, T, H), np.float32)
    for b in range(B):
        ot = np.asarray(res.results[b]["ot"], np.float32)   # [H, T]
        s = np.asarray(res.results[b]["sums"], np.float32)  # [1, T]
        out[b] = (ot / s).T
    return out


# revision 17
# speedup vs baseline: 1.1381x; 1.0332x over previous
"""Single-head causal attention (B=8, T=2048, D=1024, H=128) on 8 TRN2 NeuronCores.

Sharding: one batch element per core (data-parallel over B).

v5 design (per core, bf16 inputs, fp32 PSUM accumulation):
  - packed weights w = [V|K|Q] d-tiles, DMA'd as three slices interleaved
    with the first x d-tiles; x^T first half streamed per d-tile, second
    half as quarter blocks ordered so the late projection groups unblock
    in need-order.
  - projections run as 512-col groups (g0..g3), each 8 d-steps of a V/K/Q
    matmul trio into three 1-bank PSUM accs. g0 tracks the HBM stream;
    g1 and chunk-3's Q run inline; the remaining groups are chopped into
    per-d closures and EMITTED AS FILLER between attention units, so the
    PE chews projection work exactly where the ACT-bound attention stretch
    would otherwise idle it, and the ACT exp stream (the attention-phase
    bottleneck at ~0.95ns/col) never waits on a monolithic proj pass.
  - attention per 512-wide q-chunk: two k-tiles per unit share a
    [128,1024] S PSUM tile; diagonal tiles write left-shifted so each
    unit's S area is contiguous -> ONE exp per unit (2c+2 ACT calls per
    chunk). Causal mask via DVE multiply on the first 128 cols of each
    diagonal region. Chunk order 0,1,3,2 with chunk-3 fed by the early Q
    pass. PV accumulates into a single PSUM bank; the O^T bf16 cast is
    emitted inline at body end (frees the bank), the denominator matmul
    one body later (its DVE-dependent wait never blocks the PE queue).
  - unnormalized O^T (bf16) + per-column sums (f32) DMA'd out; the host
    divides and transposes.
  - 40 warmup matmuls bridge the DMA lead-in so the HAM un-throttles the
    PE clock before real work starts and never re-throttles.
  - PSUM banks: S-ring 2x[128,1024]=4, vacc/kacc/qacc 3 (also host the
    V-transpose tiles and denominator rows), otp 1 -> exactly 8.
"""
import numpy as np

B, T, D, H = 8, 2048, 1024, 128
ND = D // 128      # 8 d-tiles
NTK = T // 128     # 16 k-tiles
NCH = T // 512     # 4 q-chunks
SCALE = float(H) ** -0.5

_CACHE = {}


def _build():
    import concourse.bass as bass  # noqa: F401
    from concourse import bacc
    import concourse.mybir as mybir
    import concourse.tile as tile
    from concourse.masks import make_identity

    f32 = mybir.dt.float32
    bf16 = mybir.dt.bfloat16

    nc = bacc.Bacc("TRN2", target_bir_lowering=False)
    xt_d = nc.dram_tensor("xt", (128, ND, T), bf16, kind="ExternalInput")
    # w[p, 8o+d, h]: o=0 V, o=1 K, o=2 Q
    w_d = nc.dram_tensor("w", (128, 3 * ND, H), bf16, kind="ExternalInput")
    ot_d = nc.dram_tensor("ot", (H, T), bf16, kind="ExternalOutput")
    sums_d = nc.dram_tensor("sums", (1, T), f32, kind="ExternalOutput")

    with tile.TileContext(nc) as tc:
        with (
            tc.tile_pool(name="sb", bufs=1) as sb,
            tc.tile_pool(name="ps", bufs=1, space="PSUM") as ps,
        ):
            # ---- constants ----
            warmsrc = sb.tile([128, 128], bf16, tag="warmsrc")
            nc.gpsimd.memset(warmsrc[:], 1.0)
            ident = sb.tile([128, 128], bf16, tag="ident")
            make_identity(nc, ident[:])
            tri32 = sb.tile([128, 128], f32, tag="tri32")
            nc.gpsimd.memset(tri32[:], 1.0)
            nc.gpsimd.affine_select(
                out=tri32[:], in_=tri32[:],
                compare_op=mybir.AluOpType.is_ge, fill=0.0,
                base=0, pattern=[[1, 128]], channel_multiplier=-1,
            )
            trimask = sb.tile([128, 128], bf16, tag="trimask")
            nc.vector.tensor_copy(trimask[:], tri32[:])
            ones_col = sb.tile([128, 1], bf16, tag="ones_col")
            nc.gpsimd.memset(ones_col[:], 1.0)
            warm = sb.tile([128, 1], bf16, tag="warm")
            nc.scalar.activation(warm[:], warmsrc[:, 0:1],
                                 mybir.ActivationFunctionType.Exp, scale=1.0)

            # ---- input DMA launches (need-order on one HW queue) ----
            w = sb.tile([128, 3 * ND, H], bf16, tag="w")
            xt = sb.tile([128, ND, T], bf16, tag="xt")
            nc.sync.dma_start(w[:, 0:8, :], w_d[:, 0:8, :])
            nc.sync.dma_start(xt[:, 0, 0:1024], xt_d[:, 0, 0:1024])
            nc.sync.dma_start(w[:, 8:16, :], w_d[:, 8:16, :])
            nc.sync.dma_start(xt[:, 1, 0:1024], xt_d[:, 1, 0:1024])
            nc.sync.dma_start(w[:, 16:24, :], w_d[:, 16:24, :])
            for d in range(2, ND):
                nc.sync.dma_start(xt[:, d, 0:1024], xt_d[:, d, 0:1024])
            nc.sync.dma_start(xt[:, 0:4, 1536:2048], xt_d[:, 0:4, 1536:2048])
            nc.sync.dma_start(xt[:, 4:8, 1536:2048], xt_d[:, 4:8, 1536:2048])
            nc.sync.dma_start(xt[:, 0:4, 1024:1536], xt_d[:, 0:4, 1024:1536])
            nc.sync.dma_start(xt[:, 4:8, 1024:1536], xt_d[:, 4:8, 1024:1536])

            # ---- PE warmup across the DMA lead-in ----
            wmm = ps.tile([128, 1024], f32, tag="ring", bufs=2, name="wmm")
            for i in range(60):
                nc.tensor.matmul(wmm[:, 0:128], warmsrc[:], warmsrc[:],
                                 start=(i == 0), stop=(i == 59))
            # second warmup chain: interleaved into the DMA-gated g0 group
            # so the HAM never sees an idle window during the stream-in
            wmm2 = ps.tile([128, 1024], f32, tag="ring", bufs=2, name="wmm2")
            _wmm2_n = [0]

            def wmm_fill(last=False):
                nc.tensor.matmul(wmm2[:, 0:128], warmsrc[:], warmsrc[:],
                                 start=(_wmm2_n[0] == 0), stop=last)
                _wmm2_n[0] += 1

            qt = sb.tile([128, T], bf16, tag="qt")
            kt = sb.tile([128, T], bf16, tag="kt")
            v = sb.tile([128, NTK, H], bf16, tag="v")
            sums_sb = sb.tile([1, T], f32, tag="sums_sb")

            OUTS = {"v": 0, "k": 1, "q": 2}
            ACCTAG = {"v": "vacc", "k": "kacc", "q": "qacc"}

            def group_accs(tlo, outs):
                return {o: ps.tile([128, 512], f32, tag=ACCTAG[o], bufs=1,
                                   name=f"{o}acc_{tlo}") for o in outs}

            def group_trio(tlo, accs, d, outs):
                for o in outs:
                    nc.tensor.matmul(
                        accs[o][:], w[:, 8 * OUTS[o] + d, :],
                        xt[:, d, tlo:tlo + 512],
                        start=(d == 0), stop=(d == ND - 1))

            def group_evac(tlo, accs, outs):
                with nc.allow_low_precision(reason="bf16 qkv"):
                    if "k" in outs:
                        nc.scalar.activation(
                            kt[:, tlo:tlo + 512], accs["k"][:],
                            mybir.ActivationFunctionType.Copy)
                    if "q" in outs:
                        nc.vector.tensor_copy(qt[:, tlo:tlo + 512],
                                              accs["q"][:])
                    if "v" in outs:
                        vt = sb.tile([128, 512], bf16, tag="vt", bufs=2,
                                     name=f"vt_{tlo}")
                        nc.vector.tensor_copy(vt[:], accs["v"][:])
                        tp = ps.tile([128, 512], bf16, tag="vacc", bufs=1,
                                     name=f"tp_{tlo}")
                        for jj in range(4):
                            nc.tensor.transpose(
                                tp[:, jj * 128:(jj + 1) * 128],
                                vt[:, jj * 128:(jj + 1) * 128], ident[:])
                        nc.vector.tensor_copy(
                            v[:, tlo // 128:tlo // 128 + 4, :], tp[:])

            def group_inline(tlo, outs=("v", "k", "q"), wmm_fill_n=0):
                accs = group_accs(tlo, outs)
                for d in range(ND):
                    group_trio(tlo, accs, d, outs)
                    for i in range(wmm_fill_n):
                        wmm_fill(last=(d == ND - 1 and i == wmm_fill_n - 1))
                group_evac(tlo, accs, outs)

            def body(c):
                nk = 4 * c + 4
                otp = ps.tile([128, 512], f32, tag="otacc", bufs=1,
                              name=f"otp{c}")
                pacc = sb.tile([128, 512], bf16, tag="pacc", bufs=3,
                               name=f"pacc{c}")
                units = [(2 * k, 2 * k + 1) for k in range(2 * c + 2)]

                def geom(j, prev_w):
                    lo = 128 * (j - 4 * c) if j >= 4 * c else 0
                    return prev_w, 512 - lo, lo

                def emit_su(u):
                    j0, j1 = units[u]
                    stp = ps.tile([128, 1024], f32, tag="ring", bufs=2,
                                  name=f"stp{c}_{u}")
                    pt = sb.tile([128, 1024], bf16, tag="pt", bufs=4,
                                 name=f"pt{c}_{u}")
                    base = 0
                    for j in (j0, j1):
                        base, wd, lo = geom(j, base)
                        nc.tensor.matmul(
                            stp[:, base:base + wd],
                            kt[:, j * 128:(j + 1) * 128],
                            qt[:, c * 512 + lo:(c + 1) * 512],
                            start=True, stop=True,
                        )
                        base += wd
                    nc.scalar.activation(
                        pt[:, 0:base], stp[:, 0:base],
                        mybir.ActivationFunctionType.Exp, scale=SCALE)
                    if j1 >= 4 * c:
                        base = 0
                        for j in (j0, j1):
                            base, wd, lo = geom(j, base)
                            nc.vector.tensor_mul(
                                pt[:, base:base + 128],
                                pt[:, base:base + 128], trimask[:])
                            base += wd
                    return pt

                def emit_pv(u, pt):
                    j0, j1 = units[u]
                    base = 0
                    for j in (j0, j1):
                        base, wd, lo = geom(j, base)
                        nc.tensor.matmul(
                            otp[:, lo:512], v[:, j, :], pt[:, base:base + wd],
                            start=(j == 0), stop=(j == nk - 1),
                        )
                        with nc.allow_low_precision(reason="bf16 denom"):
                            if j == 0:
                                nc.vector.tensor_copy(pacc[:], pt[:, 0:512])
                            else:
                                nc.vector.tensor_add(
                                    pacc[:, lo:512], pacc[:, lo:512],
                                    pt[:, base:base + wd])
                        base += wd

                U = len(units)
                pts = {}
                for u in range(min(2, U)):
                    pts[u] = emit_su(u)
                for u in range(U):
                    if u + 2 < U:
                        pts[u + 2] = emit_su(u + 2)
                    emit_pv(u, pts.pop(u))
                # inline: cast O^T to bf16 (frees the single otp bank) + DMA
                ot_sb = sb.tile([128, 512], bf16, tag="otsb", bufs=2,
                                name=f"otsb{c}")
                with nc.allow_low_precision(reason="bf16 unnormalized out"):
                    nc.vector.tensor_copy(ot_sb[:], otp[:])
                nc.sync.dma_start(ot_d[:, c * 512:(c + 1) * 512], ot_sb[:])
                return pacc

            def tail_sums(c, pacc):
                """Denominator for chunk c, emitted one body late so its
                wait on the DVE pacc adds never blocks the PE queue."""
                sums = ps.tile([1, 512], f32, tag="kacc", bufs=1,
                               name=f"sums{c}")
                nc.tensor.matmul(sums[:], ones_col[:], pacc[:],
                                 start=True, stop=True)
                nc.vector.tensor_copy(sums_sb[:, c * 512:(c + 1) * 512],
                                      sums[:])

            # ---- schedule ----
            group_inline(0, wmm_fill_n=2)     # g0: DMA-gated, HAM kept busy
            group_inline(512)                 # g1
            # the rest of the projection is emitted at heavily DELAYED
            # priority: the scheduler weaves these matmuls into PE idle
            # slots of the ACT-bound attention stretch, and their evac
            # copies can never preempt the exp stream on ACT/DVE
            with tc.high_priority(offset=-1000000):
                group_inline(1536, outs=("q",))   # Q chunk 3 early
                group_inline(1536, outs=("v", "k"))
                group_inline(1024)
            p0 = body(0)
            p1 = body(1)
            tail_sums(0, p0)
            p3 = body(3)
            tail_sums(1, p1)
            p2 = body(2)
            tail_sums(3, p3)
            tail_sums(2, p2)
            nc.sync.dma_start(sums_d[:], sums_sb[:])

    nc.compile()
    return nc


def _in_maps(x, W_Q, W_K, W_V):
    import ml_dtypes

    bf16 = ml_dtypes.bfloat16

    def warr(W):
        return np.asarray(W, np.float32).reshape(ND, 128, H).transpose(1, 0, 2)

    wr = np.ascontiguousarray(
        np.concatenate([warr(W_V), warr(W_K), warr(W_Q)], axis=1)
    ).astype(bf16)
    x = np.asarray(x, np.float32)
    return [
        {"xt": np.ascontiguousarray(
            x[b].T.reshape(ND, 128, T).transpose(1, 0, 2)).astype(bf16),
         "w": wr}
        for b in range(B)
    ]


def _run(inputs, **kw):
    from concourse import bass_utils

    if "nc" not in _CACHE:
        _CACHE["nc"] = _build()
    return bass_utils.run_bass_kernel_spmd(
        _CACHE["nc"], _in_maps(**inputs), core_ids=list(range(B)), **kw)


def kernel(x, W_Q, W_K, W_V):
    res = _run({"x": x, "W_Q": W_Q, "W_K": W_K, "W_V": W_V})
    out = np.empty((B, T, H), np.float32)
    for b in range(B):
        ot = np.asarray(res.results[b]["ot"], np.float32)   # [H, T]
        s = np.asarray(res.results[b]["sums"], np.float32)  # [1, T]
        out[b] = (ot / s).T
    return out


# revision 20
# speedup vs baseline: 1.1638x; 1.0226x over previous
"""Single-head causal attention (B=8, T=2048, D=1024, H=128) on 8 TRN2 NeuronCores.

Sharding: one batch element per core (data-parallel over B).

v5 design (per core, bf16 inputs, fp32 PSUM accumulation):
  - packed weights w = [V|K|Q] d-tiles, DMA'd as three slices interleaved
    with the first x d-tiles; x^T first half streamed per d-tile, second
    half as quarter blocks ordered so the late projection groups unblock
    in need-order.
  - projections run as 512-col groups (g0..g3), each 8 d-steps of a V/K/Q
    matmul trio into three 1-bank PSUM accs. g0 tracks the HBM stream;
    g1 and chunk-3's Q run inline; the remaining groups are chopped into
    per-d closures and EMITTED AS FILLER between attention units, so the
    PE chews projection work exactly where the ACT-bound attention stretch
    would otherwise idle it, and the ACT exp stream (the attention-phase
    bottleneck at ~0.95ns/col) never waits on a monolithic proj pass.
  - attention per 512-wide q-chunk: two k-tiles per unit share a
    [128,1024] S PSUM tile; diagonal tiles write left-shifted so each
    unit's S area is contiguous -> ONE exp per unit (2c+2 ACT calls per
    chunk). Causal mask via DVE multiply on the first 128 cols of each
    diagonal region. Chunk order 0,1,3,2 with chunk-3 fed by the early Q
    pass. PV accumulates into a single PSUM bank; the O^T bf16 cast is
    emitted inline at body end (frees the bank), the denominator matmul
    one body later (its DVE-dependent wait never blocks the PE queue).
  - unnormalized O^T (bf16) + per-column sums (f32) DMA'd out; the host
    divides and transposes.
  - 40 warmup matmuls bridge the DMA lead-in so the HAM un-throttles the
    PE clock before real work starts and never re-throttles.
  - PSUM banks: S-ring 2x[128,1024]=4, vacc/kacc/qacc 3 (also host the
    V-transpose tiles and denominator rows), otp 1 -> exactly 8.
"""
import numpy as np

B, T, D, H = 8, 2048, 1024, 128
ND = D // 128      # 8 d-tiles
NTK = T // 128     # 16 k-tiles
NCH = T // 512     # 4 q-chunks
SCALE = float(H) ** -0.5

_CACHE = {}


def _build():
    import concourse.bass as bass  # noqa: F401
    from concourse import bacc
    import concourse.mybir as mybir
    import concourse.tile as tile
    from concourse.masks import make_identity

    f32 = mybir.dt.float32
    bf16 = mybir.dt.bfloat16

    nc = bacc.Bacc("TRN2", target_bir_lowering=False)
    xt_d = nc.dram_tensor("xt", (128, ND, T), bf16, kind="ExternalInput")
    # w[p, 8o+d, h]: o=0 V, o=1 K, o=2 Q
    w_d = nc.dram_tensor("w", (128, 3 * ND, H), bf16, kind="ExternalInput")
    ot_d = nc.dram_tensor("ot", (H, T), bf16, kind="ExternalOutput")
    sums_d = nc.dram_tensor("sums", (1, T), f32, kind="ExternalOutput")

    with tile.TileContext(nc) as tc:
        with (
            tc.tile_pool(name="sb", bufs=1) as sb,
            tc.tile_pool(name="ps", bufs=1, space="PSUM") as ps,
        ):
            # ---- constants ----
            warmsrc = sb.tile([128, 128], bf16, tag="warmsrc")
            nc.gpsimd.memset(warmsrc[:], 1.0)
            ident = sb.tile([128, 128], bf16, tag="ident")
            make_identity(nc, ident[:])
            tri32 = sb.tile([128, 128], f32, tag="tri32")
            nc.gpsimd.memset(tri32[:], 1.0)
            nc.gpsimd.affine_select(
                out=tri32[:], in_=tri32[:],
                compare_op=mybir.AluOpType.is_ge, fill=0.0,
                base=0, pattern=[[1, 128]], channel_multiplier=-1,
            )
            trimask = sb.tile([128, 128], bf16, tag="trimask")
            nc.vector.tensor_copy(trimask[:], tri32[:])
            ones_col = sb.tile([128, 1], bf16, tag="ones_col")
            nc.gpsimd.memset(ones_col[:], 1.0)
            warm = sb.tile([128, 1], bf16, tag="warm")
            nc.scalar.activation(warm[:], warmsrc[:, 0:1],
                                 mybir.ActivationFunctionType.Exp, scale=1.0)

            # ---- input DMA launches (need-order on one HW queue) ----
            w = sb.tile([128, 3 * ND, H], bf16, tag="w")
            xt = sb.tile([128, ND, T], bf16, tag="xt")
            nc.sync.dma_start(w[:, 0:8, :], w_d[:, 0:8, :])
            nc.sync.dma_start(xt[:, 0, 0:1024], xt_d[:, 0, 0:1024])
            nc.sync.dma_start(w[:, 8:16, :], w_d[:, 8:16, :])
            nc.sync.dma_start(xt[:, 1, 0:1024], xt_d[:, 1, 0:1024])
            nc.sync.dma_start(w[:, 16:24, :], w_d[:, 16:24, :])
            for d in range(2, ND):
                nc.sync.dma_start(xt[:, d, 0:1024], xt_d[:, d, 0:1024])
            nc.sync.dma_start(xt[:, 0:4, 1536:2048], xt_d[:, 0:4, 1536:2048])
            nc.sync.dma_start(xt[:, 4:8, 1536:2048], xt_d[:, 4:8, 1536:2048])
            nc.sync.dma_start(xt[:, 0:4, 1024:1536], xt_d[:, 0:4, 1024:1536])
            nc.sync.dma_start(xt[:, 4:8, 1024:1536], xt_d[:, 4:8, 1024:1536])

            # ---- PE warmup across the DMA lead-in ----
            wmm = ps.tile([128, 1024], f32, tag="ring", bufs=2, name="wmm")
            for i in range(40):
                nc.tensor.matmul(wmm[:, 0:128], warmsrc[:], warmsrc[:],
                                 start=(i == 0), stop=(i == 39))
            # second warmup chain: interleaved into the DMA-gated g0 group
            # so the HAM never sees an idle window during the stream-in
            wmm2 = ps.tile([128, 1024], f32, tag="ring", bufs=2, name="wmm2")
            _wmm2_n = [0]

            def wmm_fill(last=False):
                nc.tensor.matmul(wmm2[:, 0:128], warmsrc[:], warmsrc[:],
                                 start=(_wmm2_n[0] == 0), stop=last)
                _wmm2_n[0] += 1

            qt = sb.tile([128, T], bf16, tag="qt")
            kt = sb.tile([128, T], bf16, tag="kt")
            v = sb.tile([128, NTK, H], bf16, tag="v")
            sums_sb = sb.tile([1, T], f32, tag="sums_sb")

            OUTS = {"v": 0, "k": 1, "q": 2}
            ACCTAG = {"v": "vacc", "k": "kacc", "q": "qacc"}

            def group_accs(tlo, outs):
                return {o: ps.tile([128, 512], f32, tag=ACCTAG[o], bufs=1,
                                   name=f"{o}acc_{tlo}") for o in outs}

            def group_trio(tlo, accs, d, outs):
                for o in outs:
                    nc.tensor.matmul(
                        accs[o][:], w[:, 8 * OUTS[o] + d, :],
                        xt[:, d, tlo:tlo + 512],
                        start=(d == 0), stop=(d == ND - 1))

            def group_evac(tlo, accs, outs):
                with nc.allow_low_precision(reason="bf16 qkv"):
                    if "k" in outs:
                        # halves: the first 2 k-tiles unblock the next
                        # attention unit earlier
                        for hh in range(2):
                            nc.scalar.activation(
                                kt[:, tlo + 256 * hh:tlo + 256 * (hh + 1)],
                                accs["k"][:, 256 * hh:256 * (hh + 1)],
                                mybir.ActivationFunctionType.Copy)
                    if "q" in outs:
                        nc.vector.tensor_copy(qt[:, tlo:tlo + 512],
                                              accs["q"][:])
                    if "v" in outs:
                        vt = sb.tile([128, 512], bf16, tag="vt", bufs=2,
                                     name=f"vt_{tlo}")
                        nc.vector.tensor_copy(vt[:], accs["v"][:])
                        tp = ps.tile([128, 512], bf16, tag="vacc", bufs=1,
                                     name=f"tp_{tlo}")
                        for jj in range(4):
                            nc.tensor.transpose(
                                tp[:, jj * 128:(jj + 1) * 128],
                                vt[:, jj * 128:(jj + 1) * 128], ident[:])
                        nc.vector.tensor_copy(
                            v[:, tlo // 128:tlo // 128 + 4, :], tp[:])

            def group_inline(tlo, outs=("v", "k", "q"), wmm_fill_n=0):
                accs = group_accs(tlo, outs)
                for d in range(ND):
                    group_trio(tlo, accs, d, outs)
                    for i in range(wmm_fill_n):
                        wmm_fill(last=(d == ND - 1 and i == wmm_fill_n - 1))
                group_evac(tlo, accs, outs)

            def body(c):
                nk = 4 * c + 4
                otp = ps.tile([128, 512], f32, tag="otacc", bufs=1,
                              name=f"otp{c}")
                pacc = sb.tile([128, 512], bf16, tag="pacc", bufs=3,
                               name=f"pacc{c}")
                units = [(2 * k, 2 * k + 1) for k in range(2 * c + 2)]

                def geom(j, prev_w):
                    lo = 128 * (j - 4 * c) if j >= 4 * c else 0
                    return prev_w, 512 - lo, lo

                def emit_su(u):
                    j0, j1 = units[u]
                    stp = ps.tile([128, 1024], f32, tag="ring", bufs=2,
                                  name=f"stp{c}_{u}")
                    pt = sb.tile([128, 1024], bf16, tag="pt", bufs=4,
                                 name=f"pt{c}_{u}")
                    base = 0
                    for j in (j0, j1):
                        base, wd, lo = geom(j, base)
                        nc.tensor.matmul(
                            stp[:, base:base + wd],
                            kt[:, j * 128:(j + 1) * 128],
                            qt[:, c * 512 + lo:(c + 1) * 512],
                            start=True, stop=True,
                        )
                        base += wd
                    nc.scalar.activation(
                        pt[:, 0:base], stp[:, 0:base],
                        mybir.ActivationFunctionType.Exp, scale=SCALE)
                    if j1 >= 4 * c:
                        base = 0
                        for j in (j0, j1):
                            base, wd, lo = geom(j, base)
                            nc.vector.tensor_mul(
                                pt[:, base:base + 128],
                                pt[:, base:base + 128], trimask[:])
                            base += wd
                    return pt

                def emit_pv(u, pt):
                    j0, j1 = units[u]
                    base = 0
                    for j in (j0, j1):
                        base, wd, lo = geom(j, base)
                        nc.tensor.matmul(
                            otp[:, lo:512], v[:, j, :], pt[:, base:base + wd],
                            start=(j == 0), stop=(j == nk - 1),
                        )
                        with nc.allow_low_precision(reason="bf16 denom"):
                            if j == 0:
                                nc.vector.tensor_copy(pacc[:], pt[:, 0:512])
                            else:
                                nc.vector.tensor_add(
                                    pacc[:, lo:512], pacc[:, lo:512],
                                    pt[:, base:base + wd])
                        base += wd

                U = len(units)
                pts = {}
                for u in range(min(2, U)):
                    pts[u] = emit_su(u)
                for u in range(U):
                    if u + 2 < U:
                        pts[u + 2] = emit_su(u + 2)
                    emit_pv(u, pts.pop(u))
                # inline: cast O^T to bf16 (frees the single otp bank) + DMA
                ot_sb = sb.tile([128, 512], bf16, tag="otsb", bufs=2,
                                name=f"otsb{c}")
                with nc.allow_low_precision(reason="bf16 unnormalized out"):
                    nc.vector.tensor_copy(ot_sb[:], otp[:])
                nc.sync.dma_start(ot_d[:, c * 512:(c + 1) * 512], ot_sb[:])
                return pacc

            def tail_sums(c, pacc):
                """Denominator for chunk c, emitted one body late so its
                wait on the DVE pacc adds never blocks the PE queue."""
                sums = ps.tile([1, 512], f32, tag="kacc", bufs=1,
                               name=f"sums{c}")
                nc.tensor.matmul(sums[:], ones_col[:], pacc[:],
                                 start=True, stop=True)
                nc.vector.tensor_copy(sums_sb[:, c * 512:(c + 1) * 512],
                                      sums[:])

            # ---- schedule ----
            group_inline(0, wmm_fill_n=2)     # g0: DMA-gated, HAM kept busy
            group_inline(512)                 # g1
            # the rest of the projection is emitted at heavily DELAYED
            # priority: the scheduler weaves these matmuls into PE idle
            # slots of the ACT-bound attention stretch, and their evac
            # copies can never preempt the exp stream on ACT/DVE
            with tc.high_priority(offset=-1000000):
                group_inline(1536, outs=("q",))   # Q chunk 3 early
                group_inline(1536, outs=("v", "k"))
                group_inline(1024)
            p0 = body(0)
            p1 = body(1)
            tail_sums(0, p0)
            p3 = body(3)
            tail_sums(1, p1)
            nc.sync.dma_start(sums_d[0, 0:1024], sums_sb[:, 0:1024])
            p2 = body(2)
            # pulled into body-2's priority range so the scheduler slots the
            # chunk-3 denominator into b2's PE stream instead of the tail
            with tc.high_priority(offset=100):
                tail_sums(3, p3)
            nc.sync.dma_start(sums_d[0, 1536:2048], sums_sb[:, 1536:2048])
            tail_sums(2, p2)
            nc.sync.dma_start(sums_d[0, 1024:1536], sums_sb[:, 1024:1536])

    nc.compile()
    return nc


def _in_maps(x, W_Q, W_K, W_V):
    import ml_dtypes

    bf16 = ml_dtypes.bfloat16

    def warr(W):
        return np.asarray(W, np.float32).reshape(ND, 128, H).transpose(1, 0, 2)

    wr = np.ascontiguousarray(
        np.concatenate([warr(W_V), warr(W_K), warr(W_Q)], axis=1)
    ).astype(bf16)
    x = np.asarray(x, np.float32)
    return [
        {"xt": np.ascontiguousarray(
            x[b].T.reshape(ND, 128, T).transpose(1, 0, 2)).astype(bf16),
         "w": wr}
        for b in range(B)
    ]


def _run(inputs, **kw):
    from concourse import bass_utils

    if "nc" not in _CACHE:
        _CACHE["nc"] = _build()
    return bass_utils.run_bass_kernel_spmd(
        _CACHE["nc"], _in_maps(**inputs), core_ids=list(range(B)), **kw)


def kernel(x, W_Q, W_K, W_V):
    res = _run({"x": x, "W_Q": W_Q, "W_K": W_K, "W_V": W_V})
    out = np.empty((B, T, H), np.float32)
    for b in range(B):
        ot = np.asarray(res.results[b]["ot"], np.float32)   # [H, T]
        s = np.asarray(res.results[b]["sums"], np.float32)  # [1, T]
        out[b] = (ot / s).T
    return out
